# revision 12
# baseline (speedup 1.0000x reference)
"""Dual-stream BERT self-attention (B=4, S=1024, H=12, DH=64) on 8 Trainium2
NeuronCores.

Sharding: core c handles batch b = c // 2 and a block of 6 heads
(h0 = 6 * (c % 2)).  No collectives: each core computes its six projections,
two score blocks, one softmax over the 2048 concatenated keys, and the
probability-weighted value sum for its (batch, head-block) slice.

v3 design (baseline 266us -> v2 187us -> this):
 - Everything bf16 (halves DMA, enables fast weight load); biases are zero in
   this workload so the bias/ones contraction row is dropped (KC=6); a host
   numpy fallback guards the general case.
 - Input DMAs spread over all three DGE queues (sync=SP HWDGE, gpsimd=SWDGE,
   scalar=ACT HWDGE for the early prologue only) so the ~8MB input stream
   lands in ~9us instead of ~36us.
 - Software-pipelined pairs: while pair p's score matmuls + exps run, the
   tensor engine also executes pair p-1's PV accumulation and pair p+1's
   projections (pair 0 overlaps the v/vo projections; pair 2's first head PV
   runs inline in its own loop via a dedicated psum pool).
 - The kernel ships ctx.T [65, S] per head (64 ctx rows + the softmax
   denominator row from the ones column of allv); the host does the final
   divide + transpose.  This removes all on-device transposes/normalize.
 - PSUM (8 banks): 2 score slots [128,1024] (4) + PV accumulator [65,1024]
   (2) + 2 work slots [128,512] (2, scoped to prologue..loop1, re-used as a
   second PV accumulator pool for loop2/epilogue).
 - ~10 warm-up matmuls on a memset tile lift the PE HAM clock gate to
   2.4 GHz during the initial DMA wait; a dummy exp preloads the ACT table.
"""

import numpy as np
import ml_dtypes

import concourse.bass as bass
import concourse.tile as tile
import concourse.mybir as mybir
from concourse.bass_utils import run_bass_kernel_spmd

F32 = mybir.dt.float32
BF16 = mybir.dt.bfloat16
AF = mybir.ActivationFunctionType

B, S, D = 4, 1024, 768
H, DH = 12, 64
HPC = 6           # heads per core
N_CORES = 8
KC = 6            # contraction chunks of 128 over D=768 (no bias row)
MW = HPC * DH     # 384: per-core projection width
SC = S // 128     # 8 s-chunks
NQ = S // 512     # 2 moving-dim halves
KCH = 2 * SC      # 16 key chunks (self ++ other)
N_WARM = 10       # warm-up matmuls to lift the HAM clock gate

_DMA_OPCODES = {"DMACopy", "DMATranspose", "Trigger"}


def _split_sync_commands(nc, max_waits=1, max_updates=1):
    """This container's walrus accepts at most one sync-wait and one
    sync-update per instruction; split extras onto same-engine nops."""
    n = [0]

    def mknop(engine, waits, updates):
        n[0] += 1
        return mybir.InstNoOp(
            name=f"syncsplit-{n[0]}",
            engine=engine,
            bass_nofuse=True,
            sync_info=mybir.SyncInfo(on_wait=waits, on_update=updates),
        )

    for f in nc.m.functions:
        for bb in f.blocks:
            out = []
            changed = False
            for inst in bb.instructions:
                si = getattr(inst, "sync_info", None)
                if si is None:
                    out.append(inst)
                    continue
                waits = list(si.on_wait or [])
                if len(waits) > max_waits:
                    changed = True
                    si.on_wait = waits[:max_waits]
                    for i in range(max_waits, len(waits), max_waits):
                        out.append(mknop(inst.engine, waits[i:i + max_waits], []))
                out.append(inst)
                ups = list(si.on_update or [])
                if len(ups) > max_updates:
                    assert inst.opcode not in _DMA_OPCODES, (
                        f"can't split updates on async op {inst.name}"
                    )
                    changed = True
                    si.on_update = ups[:max_updates]
                    for i in range(max_updates, len(ups), max_updates):
                        out.append(mknop(inst.engine, [], ups[i:i + max_updates]))
            if changed:
                bb.instructions[:] = out


class CompatTileContext(tile.TileContext):
    def __exit__(self, exc_type, exc_val, exc_tb):
        r = super().__exit__(exc_type, exc_val, exc_tb)
        if exc_type is None:
            _split_sync_commands(self.nc)
        return r


def _build(repeat=1):
    nc = bass.Bass("TRN2", target_bir_lowering=False, debug=False,
                   enable_asserts=True, num_devices=1)

    xt_d = nc.dram_tensor("xt", [128, KC * S], BF16, kind="ExternalInput").ap()
    xot_d = nc.dram_tensor("xot", [128, KC * S], BF16, kind="ExternalInput").ap()
    w_d = {
        ty: nc.dram_tensor(f"w{ty}", [128, KC * MW], BF16, kind="ExternalInput").ap()
        for ty in ("q", "k", "qo", "ko", "v", "vo")
    }
    mask_d = nc.dram_tensor("mask", [128, SC], F32, kind="ExternalInput").ap()
    negb_d = nc.dram_tensor("negb", [128, 1], F32, kind="ExternalInput").ap()
    onec_d = nc.dram_tensor("onec", [128, HPC * KCH], BF16,
                            kind="ExternalInput").ap()
    # per-head transposed context [64 ctx rows + denominator row, S]
    out_d = nc.dram_tensor("out", [HPC, DH + 1, S], F32,
                           kind="ExternalOutput").ap()

    with CompatTileContext(nc) as tc:
      for _rep in range(repeat):
        with (
            tc.tile_pool(name="io", bufs=1) as io,       # persistent inputs
            tc.tile_pool(name="proj", bufs=1) as proj,   # allv
            tc.tile_pool(name="ptp", bufs=2) as ptp,     # q/k/qo/ko projections
            tc.tile_pool(name="wstr", bufs=2) as wstr,   # streamed weight slices
            tc.tile_pool(name="etp", bufs=36) as etp,    # exp(score) tiles
            tc.tile_pool(name="ctp", bufs=2) as ctp,     # ctx.T staging
            tc.tile_pool(name="scp", bufs=2, space="PSUM") as scp,   # 4 banks
            tc.tile_pool(name="pvp", bufs=1, space="PSUM") as pvp,   # 2 banks
        ):
            wkp_cm = tc.tile_pool(name="wkp", bufs=2, space="PSUM")  # 2 banks
            wkp = wkp_cm.__enter__()

            # ---- small constants + warm-up source --------------------
            warm_t = io.tile([128, 512], BF16, tag="warm")
            nc.gpsimd.memset(warm_t[:], 0.25)
            mask_t = io.tile([128, SC], F32, tag="mask")
            nc.scalar.dma_start(mask_t[:], mask_d[:])
            negb_t = io.tile([128, 1], F32, tag="negb")
            nc.scalar.dma_start(negb_t[:], negb_d[:])
            scratch_t = io.tile([128, 1], BF16, tag="scratch")
            # dummy exp: pull the ACT function table load off the critical path
            nc.scalar.activation(scratch_t[:], negb_t[:], AF.Exp)

            # allv[(p)art=key, head, chunk, dh|1]: value rows + ones column
            allv = proj.tile([128, HPC, KCH, DH + 1], BF16, tag="av")
            onec_t = io.tile([128, HPC * KCH], BF16, tag="onec")

            # ---- bulk input streams over three DGE queues ------------
            xt = io.tile([128, KC * S], BF16, tag="xt")
            xot = io.tile([128, KC * S], BF16, tag="xot")
            wv_t = io.tile([128, KC * MW], BF16, tag="wv")
            wvo_t = io.tile([128, KC * MW], BF16, tag="wvo")

            wslice = {}

            def fetch_pair_weights(p, tys, eng=None):
                for ty in tys:
                    wt = wstr.tile([128, KC, 128], BF16, tag=f"w{ty}",
                                   name=f"w_{ty}{p}")
                    e = eng or (nc.gpsimd if ty in ("qo", "ko") else nc.sync)
                    e.dma_start(
                        wt[:],
                        w_d[ty].rearrange("q (k m) -> q k m", k=KC)
                            [:, :, 128 * p:128 * p + 128],
                    )
                    wslice[(ty, p)] = wt

            def xchunk(dst, src, k, eng):
                eng.dma_start(dst[:, S * k:S * k + S], src[:, S * k:S * k + S])

            # Queue plan (first-exp needs xt + wq0/wk0; scalar only carries
            # part of that critical prefix, then stays clear for exp):
            #  scalar: mask negb xt1 xt3 xt5 wk0
            #  sync:   xt0 xt2 xt4 wq0 wq1 wk1 wv wq2 wk2 wvo
            #  gpsimd: xot0 xot1 wqo0 xot2 xot3 wko0 xot4 xot5 wqo1 wko1
            #          onec allv-ones wqo2 wko2
            xchunk(xt, xt_d, 0, nc.sync)
            xchunk(xot, xot_d, 0, nc.gpsimd)
            xchunk(xt, xt_d, 1, nc.scalar)
            xchunk(xot, xot_d, 1, nc.gpsimd)
            xchunk(xt, xt_d, 2, nc.sync)
            fetch_pair_weights(0, ("qo",))
            xchunk(xt, xt_d, 3, nc.scalar)
            xchunk(xot, xot_d, 2, nc.gpsimd)
            xchunk(xt, xt_d, 4, nc.sync)
            xchunk(xot, xot_d, 3, nc.gpsimd)
            fetch_pair_weights(0, ("q",))
            fetch_pair_weights(0, ("ko",))
            xchunk(xt, xt_d, 5, nc.scalar)
            xchunk(xot, xot_d, 4, nc.gpsimd)
            fetch_pair_weights(0, ("k",), eng=nc.scalar)
            xchunk(xot, xot_d, 5, nc.gpsimd)
            # The rest of the input stream (wv/wvo, pair-1/2 weights, onec)
            # is emitted inside loop-0's windows: emitting 30+ DMAs up front
            # recycles the 8 DMA-completion semaphore lanes ahead of their
            # compute waiters, which then stall on a much later transfer.

            # ---- warm-up matmuls (lift HAM clock gate during DMA wait)
            wps = wkp.tile([128, 512], F32, tag="wk", name="warmps")
            for i in range(N_WARM):
                nc.tensor.matmul(wps[:], warm_t[:, 0:128], warm_t[:],
                                 start=True, stop=True)

            pt = {ty: [None] * 3 for ty in ("q", "k", "qo", "ko")}

            def proj_task(ty, p, nh):
                """One projection psum group: out pt[ty][p][:, nh*512:...]."""
                if pt[ty][p] is None:
                    pt[ty][p] = ptp.tile([128, S], BF16, tag=f"pt{ty}",
                                         name=f"pt_{ty}{p}")
                wt = wslice[(ty, p)]
                src = xot if ty == "ko" else xt
                ps = wkp.tile([128, 512], F32, tag="wk", name=f"pps_{ty}{p}{nh}")
                for k in range(KC):
                    nc.tensor.matmul(
                        ps[:],
                        wt[:, k, :],
                        src[:, S * k + 512 * nh: S * k + 512 * nh + 512],
                        start=(k == 0), stop=(k == KC - 1),
                    )
                nc.vector.tensor_copy(
                    pt[ty][p][:, 512 * nh:512 * nh + 512], ps[:])

            def v_task(ti, sc):
                """One v/vo projection psum group -> allv columns."""
                ty, wt, src = (("v", wv_t, xt), ("vo", wvo_t, xot))[ti]
                ps = wkp.tile([128, 512], F32, tag="wk", name=f"vps_{ty}{sc}")
                for k in range(KC):
                    nc.tensor.matmul(
                        ps[:, 0:MW],
                        src[:, S * k + 128 * sc: S * k + 128 * sc + 128],
                        wt[:, MW * k: MW * k + MW],
                        start=(k == 0), stop=(k == KC - 1),
                    )
                nc.vector.tensor_copy(
                    allv[:, :, SC * ti + sc, 0:DH],
                    ps[:, 0:MW].rearrange("p (h d) -> p h d", d=DH),
                )

            # prologue: pair-0 q/k projections only (all the self-side scores
            # need); qo/ko ride loop-0's early windows so exp #1 isn't queued
            # behind them
            for ty, nh in (("q", 0), ("k", 0), ("q", 1), ("k", 1)):
                proj_task(ty, 0, nh)

            # ---- software pipeline over pairs ------------------------
            et_tiles = {}

            def emit_scores_exp(p, c, hh):
                self_side = c < SC
                kt_src = pt["k" if self_side else "ko"][p]
                qt_src = pt["q" if self_side else "qo"][p]
                rows = slice(64 * hh, 64 * hh + 64)
                col = 128 * (c % SC)
                sc_t = scp.tile([128, S], F32, tag="sc", name=f"sc{p}_{c}_{hh}")
                for nh in range(NQ):
                    nc.tensor.matmul(
                        sc_t[:, 512 * nh:512 * nh + 512],
                        kt_src[rows, col:col + 128],
                        qt_src[rows, 512 * nh:512 * nh + 512],
                        start=True, stop=True,
                    )
                et_t = etp.tile([128, S], BF16, tag="et", name=f"et{p}_{c}_{hh}")
                bias = mask_t[:, c:c + 1] if self_side else negb_t[:]
                nc.scalar.activation(et_t[:], sc_t[:], AF.Exp, bias=bias,
                                     scale=float(1.0 / np.sqrt(DH)))
                et_tiles[(p, hh, c)] = et_t

            pv_state = {}

            def pv_step(pool, p, hh, kc):
                """Two accumulating PV matmuls (s-halves) for head hh of
                pair p, key chunk kc."""
                h = 2 * p + hh
                if kc == 0:
                    pv_state[(p, hh)] = pool.tile(
                        [DH + 1, S], F32, tag="pv", name=f"pv{p}_{hh}")
                pv = pv_state[(p, hh)]
                et_t = et_tiles.pop((p, hh, kc))
                for sh in range(NQ):
                    nc.tensor.matmul(
                        pv[:, 512 * sh:512 * sh + 512],
                        allv[:, h, kc, :],
                        et_t[:, 512 * sh:512 * sh + 512],
                        start=(kc == 0), stop=(kc == KCH - 1),
                    )

            def pv_finish(p, hh):
                """Dump pv psum to SBUF and DMA the [65, S] block out."""
                pv = pv_state.pop((p, hh))
                h = 2 * p + hh
                ct = ctp.tile([DH + 1, S], F32, tag="ct", name=f"ct{p}_{hh}")
                nc.vector.tensor_copy(ct[:], pv[:])
                eng = nc.sync if hh == 0 else nc.gpsimd
                eng.dma_start(out_d[h], ct[:])

            # ---- loop 0: scores/exp pair 0; bg: p0 qo/ko, p1 proj, v;
            # late DMA emissions ride the windows so semaphore lanes are
            # not recycled ahead of their waiters
            loop0_bg = [("qo", 0, 0), ("qo", 0, 1),
                        ("ko", 0, 0), ("ko", 0, 1),
                        ("q", 1, 0), ("k", 1, 0),
                        ("q", 1, 1), ("k", 1, 1),
                        ("qo", 1, 0), ("ko", 1, 0),
                        ("qo", 1, 1), ("ko", 1, 1)]

            def loop0_dma(c):
                if c == 0:
                    fetch_pair_weights(1, ("q",))
                elif c == 1:
                    fetch_pair_weights(1, ("k",))
                elif c == 2:
                    nc.sync.dma_start(wv_t[:], w_d["v"][:])
                elif c == 3:
                    fetch_pair_weights(1, ("qo",))
                elif c == 4:
                    fetch_pair_weights(1, ("ko",))
                elif c == 6:
                    nc.sync.dma_start(wvo_t[:], w_d["vo"][:])
                elif c == 8:
                    fetch_pair_weights(2, ("q",))
                elif c == 9:
                    fetch_pair_weights(2, ("k",))
                elif c == 10:
                    nc.gpsimd.dma_start(onec_t[:], onec_d[:])
                    nc.gpsimd.dma_start(
                        allv[:, :, :, DH:DH + 1],
                        onec_t[:].rearrange("p (h c o) -> p h c o",
                                            c=KCH, o=1),
                    )
                elif c == 12:
                    fetch_pair_weights(2, ("qo",))
                elif c == 13:
                    fetch_pair_weights(2, ("ko",))

            for c in range(KCH):
                loop0_dma(c)
                emit_scores_exp(0, c, 0)
                emit_scores_exp(0, c, 1)
                if c < len(loop0_bg):
                    ty, p, nh = loop0_bg[c]
                    proj_task(ty, p, nh)
                if c >= SC:
                    v_task(0, c - SC)

            # ---- loop 1: pair 1 scores/exp + pair-0 PV + vo + p2 q/k -
            for c in range(KCH):
                emit_scores_exp(1, c, 0)
                emit_scores_exp(1, c, 1)
                if c < SC:
                    v_task(1, c)
                hh, base = (0, 0) if c < SC else (1, SC)
                pv_step(pvp, 0, hh, 2 * (c - base))
                pv_step(pvp, 0, hh, 2 * (c - base) + 1)
                if c - base == SC - 1:
                    pv_finish(0, hh)
                if SC <= c < SC + 4:
                    ty, nh = (("q", 0), ("k", 0), ("q", 1), ("k", 1))[c - SC]
                    proj_task(ty, 2, nh)

            # ---- loop 2: pair 2 scores/exp + pair-1 PV + p2 qo/ko
            # projections (first 4 windows, wkp), then wkp becomes the
            # second PV accumulator pool for inline pair-2 h0 PV
            pvp2 = None
            pvp2_cm = None
            for c in range(KCH):
                emit_scores_exp(2, c, 0)
                emit_scores_exp(2, c, 1)
                if c < 4:
                    ty, nh = (("qo", 0), ("ko", 0), ("qo", 1), ("ko", 1))[c]
                    proj_task(ty, 2, nh)
                if c == 4:
                    wkp_cm.__exit__(None, None, None)
                    pvp2_cm = tc.tile_pool(name="pvp2", bufs=1, space="PSUM")
                    pvp2 = pvp2_cm.__enter__()
                hh, base = (0, 0) if c < SC else (1, SC)
                pv_step(pvp, 1, hh, 2 * (c - base))
                pv_step(pvp, 1, hh, 2 * (c - base) + 1)
                if c - base == SC - 1:
                    pv_finish(1, hh)
                if c >= 5:
                    pv_step(pvp2, 2, 0, c - 5)  # inline kc 0..10

            # ---- epilogue: inline kc tail + pair-2 h1 PV -------------
            for kc in range(KCH - 5, KCH):
                pv_step(pvp2, 2, 0, kc)
            pv_finish(2, 0)
            for kc in range(KCH):
                pv_step(pvp, 2, 1, kc)
            pv_finish(2, 1)

            pvp2_cm.__exit__(None, None, None)

    return nc


def _to_chunked(a, ncols):
    """[KC*128, ncols] -> [128, KC*ncols] with chunk c at cols [c*ncols, ...)."""
    return np.ascontiguousarray(
        a.reshape(KC, 128, ncols).transpose(1, 0, 2).reshape(128, KC * ncols)
    )


def _shard_inputs(hidden_states, hidden_states_other, attention_mask,
                  Wq, bq, Wk, bk, Wv, bv, Wqo, bqo, Wko, bko, Wvo, bvo):
    f32 = np.float32
    bf16 = ml_dtypes.bfloat16
    hs = np.asarray(hidden_states, f32)
    hso = np.asarray(hidden_states_other, f32)
    am = np.asarray(attention_mask, f32)
    ws = {"q": Wq, "k": Wk, "qo": Wqo, "ko": Wko, "v": Wv, "vo": Wvo}

    onec = np.ones((128, HPC * KCH), bf16)
    negb = np.full((128, 1), -4.0, f32)

    in_maps = []
    for core in range(N_CORES):
        b, hh = core // 2, core % 2
        m = {}
        for name, x in (("xt", hs[b]), ("xot", hso[b])):
            m[name] = _to_chunked(
                np.ascontiguousarray(x.T), S).astype(bf16)
        sl = slice(MW * hh, MW * hh + MW)
        for ty, W in ws.items():
            m[f"w{ty}"] = _to_chunked(
                np.ascontiguousarray(np.asarray(W, f32)[sl].T), MW).astype(bf16)
        m["mask"] = np.ascontiguousarray(am[b, 0, 0].reshape(SC, 128).T) - 4.0
        m["negb"] = negb
        m["onec"] = onec
        in_maps.append(m)
    return in_maps


def _numpy_reference(hidden_states, hidden_states_other, attention_mask,
                     Wq, bq, Wk, bk, Wv, bv, Wqo, bqo, Wko, bko, Wvo, bvo):
    """Exact fallback for the (never-hit) nonzero-bias case."""
    f = np.float32

    def split_heads(x):
        Bb, Ss, _ = x.shape
        return x.reshape(Bb, Ss, H, DH).transpose(0, 2, 1, 3)

    lin = lambda x, W, b: x @ np.asarray(W, f).T + np.asarray(b, f)
    hs = np.asarray(hidden_states, f)
    hso = np.asarray(hidden_states_other, f)
    q = split_heads(lin(hs, Wq, bq))
    k = split_heads(lin(hs, Wk, bk))
    v = split_heads(lin(hs, Wv, bv))
    qo = split_heads(lin(hs, Wqo, bqo))
    ko = split_heads(lin(hso, Wko, bko))
    vo = split_heads(lin(hso, Wvo, bvo))
    scale = 1.0 / np.sqrt(DH)
    s1 = np.einsum('bhqd,bhkd->bhqk', q, k) * scale + np.asarray(attention_mask, f)
    s2 = np.einsum('bhqd,bhkd->bhqk', qo, ko) * scale
    alls = np.concatenate([s1, s2], axis=-1)
    alls -= alls.max(axis=-1, keepdims=True)
    p = np.exp(alls)
    p /= p.sum(axis=-1, keepdims=True)
    ctx = np.einsum('bhqk,bhkd->bhqd', p, np.concatenate([v, vo], axis=-2))
    Bb = ctx.shape[0]
    return ctx.transpose(0, 2, 1, 3).reshape(Bb, S, H * DH).astype(f)


_NC_CACHE = {}


def _get_nc(repeat=1):
    if repeat not in _NC_CACHE:
        _NC_CACHE[repeat] = _build(repeat)
    return _NC_CACHE[repeat]


def kernel(**inputs):
    if any(np.any(np.asarray(inputs[k])) for k in
           ("bq", "bk", "bv", "bqo", "bko", "bvo")):
        return _numpy_reference(**inputs)
    in_maps = _shard_inputs(**inputs)
    nc = _get_nc()
    res = run_bass_kernel_spmd(nc, in_maps, core_ids=list(range(N_CORES)))
    out = np.empty((B, S, D), np.float32)
    for core in range(N_CORES):
        b, hh = core // 2, core % 2
        o = res.results[core]["out"]           # [HPC, DH+1, S]
        ctx = o[:, 0:DH, :] / o[:, DH:DH + 1, :]
        out[b, :, MW * hh:MW * hh + MW] = (
            ctx.transpose(2, 0, 1).reshape(S, MW))
    return out
